# revision 45
# baseline (speedup 1.0000x reference)
"""Trainium2 Bass kernel for nn_AttentionSampling (sparse window attention block).

Sharding: 8 cores, data-parallel, 1024 windows (half a batch) per core; windows are
independent so there is no cross-core communication. Activations live in a transposed
[d, tokens] layout (host pre-transposes q/k) so projections run weight-stationary.

Precision: q/k/ffn projections run fp8e4 DoubleRow (weights host-prescaled x16/x64,
folded back via ACT scale; ffn2's 1/64 rides the residual — LN2 is scale-invariant).
Scores, value path and LN broadcasts are bf16; LN stats accumulate in fp32 PSUM.

Value path (exact algebra): the windowed weighted-sum commutes with the value
projection, so the kernel downsamples RAW value first — masked scores are
transposed (PE) into the banded [keys, windows] operand; each (key-chunk, d-tile)
pair is a single 32-col matmul since chunk c only feeds windows 32c..32c+31 —
then projects the [512, 128]-shrunk vs through w_v (4 matmuls vs 16).

Pipeline: block b's value path (PE work gated on DVE mask + ACT copies) is
emitted after block b+1's k-proj/scores so the PE never stalls on them;
residual-transpose + LN trail two blocks behind.
"""

import sys
import types

# If BASS_TRACE is set in an environment whose antenv package lacks
# axon_hooks, run_bass_kernel_spmd would crash on import; provide a stub
# (a None hook makes bass_utils skip tracing gracefully).
try:
    import antenv.axon_hooks  # noqa: F401
except ImportError:
    _m = types.ModuleType("antenv.axon_hooks")
    _m.get_axon_ntff_profile_hook = lambda: None
    _m.set_axon_ntff_profile_hook = lambda h: None
    sys.modules["antenv.axon_hooks"] = _m
    try:
        import antenv

        antenv.axon_hooks = _m
    except ImportError:
        pass

import contextlib

import numpy as np

import concourse.bass as bass
import concourse.bacc as bacc_mod
import concourse.mybir as mybir
import concourse.tile as tile
from concourse.bass import ts, ds
from concourse.bass_utils import run_bass_kernel_spmd

FP32 = mybir.dt.float32
FP32R = mybir.dt.float32r
FP8 = mybir.dt.float8e4
AF = mybir.ActivationFunctionType
OP = mybir.AluOpType
DR = mybir.MatmulPerfMode.DoubleRow

MM_DT = mybir.dt.bfloat16  # matmul operands; attention weights/LN stay fp32
# fp8 weight pre-scales (host multiplies weights up so fp8 stays in normal
# range; the ACT after each matmul folds the inverse back in)
WQK_SCALE = 16.0
FFN_SCALE = 64.0

B, SQ, SK, D, F = 4, 2048, 8192, 512, 4
NCORES = 8
WPC = B * SQ // NCORES        # 1024 windows (= tokens) per core
KPC = WPC * F                 # 4096 keys per core
NBLK = WPC // 128             # 8 attention blocks: 128 windows / 512 keys
NSB = WPC // 512              # 2 superblocks of 512 tokens
DT = D // 128                 # 4 d-tiles
EPS = 1e-5

_CACHE = {}


def _emit_ln_T(nc, P, resid_view, sq_tile, stats_sb, out_cb, n=512):
    """Transposed LayerNorm over D for an n-token chunk.

    resid_view/sq_tile: [128, DT, n]; sq_tile doubles as apply scratch.
    stats_sb: [1, 1024] (mean at 0, rstd at 512, each n long).
    out_cb(dt, src): write normalized+affine output for d-tile dt from src.
    """
    mean = stats_sb[:, :n]          # bf16 (bc matmul rhs streams 1 cyc/row)
    rstd_bf = stats_sb[:, 512 : 512 + n]

    nc.vector.tensor_tensor(sq_tile[:], resid_view, resid_view, op=OP.mult)

    # resid/sq tiles are bf16 so the stats matmuls stream at 1 cyc/row
    st_sum = P["st"].tile([1, 512], FP32, tag="st", name="st_sum")[:, :n]
    for dt in range(DT):
        nc.tensor.matmul(
            st_sum, lhsT=P["ones_col"], rhs=resid_view[:, dt, :],
            start=(dt == 0), stop=(dt == DT - 1),
        )
    nc.scalar.activation(out=mean, in_=st_sum, func=AF.Copy, scale=1.0 / D)

    st_sq = P["st"].tile([1, 512], FP32, tag="st", name="st_sq")[:, :n]
    for dt in range(DT):
        nc.tensor.matmul(
            st_sq, lhsT=P["ones_col"], rhs=sq_tile[:, dt, :],
            start=(dt == 0), stop=(dt == DT - 1),
        )
    e2 = P["small"].tile([1, 512], FP32, tag="e2", name="e2")[:, :n]
    var = P["small"].tile([1, 512], FP32, tag="var", name="var")[:, :n]
    nc.scalar.activation(out=e2, in_=st_sq, func=AF.Copy, scale=1.0 / D)
    nc.vector.tensor_tensor(var, mean, mean, op=OP.mult)
    nc.vector.tensor_tensor(var, e2, var, op=OP.subtract)
    nc.scalar.activation(out=var, in_=var, func=AF.Sqrt, bias=P["eps_t"], scale=1.0)
    rstd32 = P["small"].tile([1, 512], FP32, tag="rstd32", name="rstd32")[:, :n]
    nc.vector.reciprocal_approx_fast(out=rstd32, in_=var)
    nc.vector.tensor_scalar_add(rstd_bf, rstd32, 0.0)

    # round 1: subtract broadcast mean (sq_tile becomes the scratch)
    bc = P["bc"].tile([128, 512], FP32, tag="bc", name="bc_mean")[:, :n]
    nc.tensor.matmul(bc, lhsT=P["ones_row"], rhs=mean, start=True, stop=True)
    for dt in range(DT):
        nc.vector.tensor_tensor(sq_tile[:, dt, :], resid_view[:, dt, :], bc, op=OP.subtract)
    # round 2: multiply broadcast rstd (in place), then affine via ACT
    bc2 = P["bc"].tile([128, 512], FP32, tag="bc", name="bc_rstd")[:, :n]
    nc.tensor.matmul(bc2, lhsT=P["ones_row"], rhs=rstd_bf, start=True, stop=True)
    for dt in range(DT):
        nc.vector.tensor_tensor(sq_tile[:, dt, :], sq_tile[:, dt, :], bc2, op=OP.mult)
        out_cb(dt, sq_tile[:, dt, :])


def build_program():
    nc = bacc_mod.Bacc(None, target_bir_lowering=False)

    qT_d = nc.dram_tensor("qT", [D, WPC], FP8, kind="ExternalInput")
    kT_d = nc.dram_tensor("kT", [D, KPC], FP8, kind="ExternalInput")
    vT_d = nc.dram_tensor("vT", [D, KPC], MM_DT, kind="ExternalInput")
    wq_d = nc.dram_tensor("w_q", [D, D], FP8, kind="ExternalInput")
    wk_d = nc.dram_tensor("w_k", [D, D], FP8, kind="ExternalInput")
    wv_d = nc.dram_tensor("w_v", [D, D], MM_DT, kind="ExternalInput")
    w1_d = nc.dram_tensor("ffn_w1", [D, D], FP8, kind="ExternalInput")
    w2_d = nc.dram_tensor("ffn_w2", [D, D], FP8, kind="ExternalInput")
    bq_d = nc.dram_tensor("b_q", [D], FP32, kind="ExternalInput")
    bk_d = nc.dram_tensor("b_k", [D], FP32, kind="ExternalInput")
    bv_d = nc.dram_tensor("b_v", [D], FP32, kind="ExternalInput")
    b1_d = nc.dram_tensor("ffn_b1", [D], FP32, kind="ExternalInput")
    b2_d = nc.dram_tensor("ffn_b2", [D], FP32, kind="ExternalInput")
    g1_d = nc.dram_tensor("ln1_g", [D], FP32, kind="ExternalInput")
    gb1_d = nc.dram_tensor("ln1_b", [D], FP32, kind="ExternalInput")
    g164_d = nc.dram_tensor("ln1_g64", [D], FP32, kind="ExternalInput")
    gb164_d = nc.dram_tensor("ln1_b64", [D], FP32, kind="ExternalInput")
    g2_d = nc.dram_tensor("ln2_g", [D], FP32, kind="ExternalInput")
    gb2_d = nc.dram_tensor("ln2_b", [D], FP32, kind="ExternalInput")
    mask_d = nc.dram_tensor("cmask", [128, 512], FP32, kind="ExternalInput")
    ident_d = nc.dram_tensor("cident", [128, 128], FP32, kind="ExternalInput")
    outT_d = nc.dram_tensor("outT", [D, WPC], FP32, kind="ExternalOutput")

    qT_t = qT_d.rearrange("(o p) n -> p o n", p=128)
    kT_t = kT_d.rearrange("(o p) n -> p o n", p=128)
    vT_t = vT_d.rearrange("(o p) n -> p o n", p=128)
    outT_t = outT_d.rearrange("(o p) n -> p o n", p=128)

    with tile.TileContext(nc) as tc, contextlib.ExitStack() as ctx:
        singles = ctx.enter_context(tc.tile_pool(name="singles", bufs=1))
        inp = ctx.enter_context(tc.tile_pool(name="inp", bufs=4))
        ktp_p = ctx.enter_context(tc.tile_pool(name="ktp", bufs=1))
        att_p = ctx.enter_context(tc.tile_pool(name="att", bufs=2))
        resid_p = ctx.enter_context(tc.tile_pool(name="resid", bufs=2))
        hT_p = ctx.enter_context(tc.tile_pool(name="hT", bufs=1))
        out_p = ctx.enter_context(tc.tile_pool(name="outp", bufs=2))
        small = ctx.enter_context(tc.tile_pool(name="small", bufs=1))
        ps_proj = ctx.enter_context(tc.tile_pool(name="ps_proj", bufs=3, space="PSUM"))
        ps_vf = ctx.enter_context(tc.tile_pool(name="ps_vf", bufs=2, space="PSUM"))
        ps_tr = ctx.enter_context(tc.tile_pool(name="ps_tr", bufs=1, space="PSUM"))
        ps_st = ctx.enter_context(tc.tile_pool(name="ps_st", bufs=1, space="PSUM"))
        ps_bc = ctx.enter_context(tc.tile_pool(name="ps_bc", bufs=1, space="PSUM"))

        def load_w(d, tg, dt_=MM_DT):
            t = singles.tile([128, DT, 512], dt_, tag=tg)
            nc.sync.dma_start(out=t, in_=d.rearrange("(o p) n -> p o n", p=128))
            return t

        def load_b(d, tg):
            t = singles.tile([128, DT], FP32, tag=tg)
            nc.sync.dma_start(out=t, in_=d.rearrange("(o p) -> p o", p=128))
            return t

        # issue order matters: Sync issues DMAs in program order, and the PE's
        # first work (q-proj then k-proj of block 0) must not wait behind a
        # dozen constant loads.
        # DMA issue order == PE consumption order: wq+q(sb0) gate the very
        # first matmul, then k0 for k-proj(0); the 2MB v0 comes later.
        wq_sb = load_w(wq_d, "wq", FP8)
        bq_sb = load_b(bq_d, "bq")
        q_ins = []
        q_in = inp.tile([128, DT, 512], FP8, tag="in_t", name="q_in0")
        nc.sync.dma_start(out=q_in, in_=qT_t[:, :, ts(0, 512)])
        q_ins.append(q_in)
        k0 = inp.tile([128, DT, 512], FP8, tag="in_t", name="k0")
        nc.sync.dma_start(out=k0, in_=kT_t[:, :, ts(0, 512)])

        wk_sb = load_w(wk_d, "wk", FP8)
        bk_sb = load_b(bk_d, "bk")
        q_in = inp.tile([128, DT, 512], FP8, tag="in_t", name="q_in1")
        nc.sync.dma_start(out=q_in, in_=qT_t[:, :, ts(1, 512)])
        q_ins.append(q_in)
        v0 = inp.tile([128, DT, 512], MM_DT, tag="in_v", bufs=2, name="v0")
        nc.sync.dma_start(out=v0, in_=vT_t[:, :, ts(0, 512)])
        kv0 = [k0, v0]
        wv_sb = load_w(wv_d, "wv")
        mask = singles.tile([128, 512], FP32, tag="mask")
        nc.sync.dma_start(out=mask, in_=mask_d[:, :])
        bv_rep = singles.tile([128, 512], FP32, tag="bv_rep")
        nc.gpsimd.dma_start(
            out=bv_rep, in_=bass.AP(tensor=bv_d, offset=0, ap=[[0, 128], [1, 512]])
        )
        identity = singles.tile([128, 128], FP32, tag="ident")
        nc.sync.dma_start(out=identity, in_=ident_d[:, :])
        g1_sb = load_b(g1_d, "g1")
        gb1_sb = load_b(gb1_d, "gb1")
        g164_sb = load_b(g164_d, "g164")
        gb164_sb = load_b(gb164_d, "gb164")
        ones_col = singles.tile([128, 1], MM_DT, tag="ones_col")
        nc.gpsimd.memset(ones_col, 1.0)
        ones_row = singles.tile([1, 128], MM_DT, tag="ones_row")
        nc.gpsimd.memset(ones_row, 1.0)
        eps_t = singles.tile([1, 1], FP32, tag="eps")
        nc.gpsimd.memset(eps_t, EPS)
        late = {}

        def load_late_consts():
            late["w1"] = load_w(w1_d, "w1", FP8)
            late["b1"] = load_b(b1_d, "b1")
            late["w2"] = load_w(w2_d, "w2", FP8)
            late["b2"] = load_b(b2_d, "b2")  # host pre-scales by FFN_SCALE
            late["g2"] = load_b(g2_d, "g2")
            late["gb2"] = load_b(gb2_d, "gb2")

        P = {
            "st": ps_st, "bc": ps_bc, "small": small,
            "ones_col": ones_col, "ones_row": ones_row, "eps_t": eps_t,
        }

        qTp = singles.tile([128, DT, WPC], MM_DT, tag="qTp")
        xT8 = singles.tile([128, DT, WPC], FP8, tag="xT8")
        xT64 = singles.tile([128, DT, WPC], MM_DT, tag="xT64")

        def proj_T(w_sb, bias_sb, in_sb, out_sb, out_col0, n):
            # fp8 DoubleRow: two 128-row k-tiles per matmul at 0.5 cyc/row
            for do in range(DT):
                ps = ps_proj.tile([128, 512], FP32, tag="proj_ps", name="proj_ps")
                ps = ps[:, :n]
                for kp in range(0, DT, 2):
                    nc.tensor.matmul(
                        ps, lhsT=w_sb[:, kp : kp + 2, ts(do, 128)],
                        rhs=in_sb[:, kp : kp + 2, :n],
                        start=(kp == 0), stop=(kp == DT - 2), perf_mode=DR,
                    )
                nc.scalar.activation(
                    out=out_sb[:, do, ds(out_col0, n)], in_=ps, func=AF.Relu,
                    bias=bias_sb[:, do : do + 1], scale=1.0 / WQK_SCALE,
                )

        # ---- phase 1: q projection of superblock 0 only; sb1 slots in
        # after k-proj(0) starts so the PE never waits on the q1 DMA ----
        proj_T(wq_sb, bq_sb, q_ins[0], qTp, 0, 512)

        # ---- phase 2: attention ----
        residT = {}  # superblock -> tile [128, DT, 512]

        def emit_front(b):
            if b == 0:
                k_in, v_in = kv0
            else:
                k_in = inp.tile([128, DT, 512], FP8, tag="in_t")
                nc.sync.dma_start(out=k_in, in_=kT_t[:, :, ts(b, 512)])
                v_in = inp.tile([128, DT, 512], MM_DT, tag="in_v", bufs=2)
                nc.sync.dma_start(out=v_in, in_=vT_t[:, :, ts(b, 512)])

            kTp = ktp_p.tile([128, DT, 512], MM_DT, tag="kTp")
            proj_T(wk_sb, bk_sb, k_in, kTp, 0, 512)

            sc_ps = ps_proj.tile([128, 512], FP32, tag="proj_ps", name="sc_ps")
            for ki in range(DT):
                nc.tensor.matmul(
                    sc_ps, lhsT=qTp[:, ki, ts(b, 128)], rhs=kTp[:, ki, :],
                    start=(ki == 0), stop=(ki == DT - 1),
                )
            sm = att_p.tile([128, 512], FP32, tag="sm")
            nc.vector.tensor_tensor(sm, sc_ps, mask, op=OP.mult)
            wts = small.tile([128, F], FP32, tag="wts")
            nc.vector.tensor_reduce(
                out=wts, in_=sm.rearrange("p (kw f) -> p f kw", f=F),
                axis=mybir.AxisListType.X, op=OP.add,
            )
            # v projection (f-strided) + fused weighted accumulation from PSUM
            acc = bv_rep
            for f in range(F):
                ps = ps_vf.tile([128, 512], FP32, tag="vf", name="vf_ps")
                for ki in range(DT):
                    nc.tensor.matmul(
                        ps, lhsT=v_in[:, ki, f::4], rhs=wv_sb[:, ki, :],
                        start=(ki == 0), stop=(ki == DT - 1),
                    )
                tg = "ao_final" if f == F - 1 else "ao_acc"
                nxt = att_p.tile([128, 512], FP32, tag=tg, name="ao_acc")
                nc.vector.scalar_tensor_tensor(
                    out=nxt, in0=ps, scalar=wts[:, f : f + 1], in1=acc,
                    op0=OP.mult, op1=OP.add,
                )
                acc = nxt
            return acc

        def emit_mid(b, acc):
            sb, col = b // 4, (b % 4) * 128
            if col == 0:
                residT[sb] = resid_p.tile([128, DT, 512], MM_DT, tag="residT", name="residT")
            r = residT[sb]
            for dt in range(DT):
                ps_t = ps_tr.tile([128, 128], FP32, tag="tr_ps", name="tr_ps")
                nc.tensor.transpose(ps_t, acc[:, ts(dt, 128)], identity)
                nc.vector.tensor_tensor(
                    r[:, dt, ds(col, 128)], ps_t, qTp[:, dt, ts(b, 128)], op=OP.add
                )

        def emit_ln1(c0, n):
            sb, loc = c0 // 512, c0 % 512
            sq = resid_p.tile([128, DT, n], MM_DT, tag="sq1")
            stats = small.tile([1, 1024], MM_DT, tag="stats1")

            def write_x(dt, src):
                # dual write: fp8 x for the ffn1 DoubleRow rhs, and bf16
                # 64*x (FFN_SCALE) so the ffn2 residual stt needs no
                # unscale op — LN2 is invariant to the uniform 64x.
                nc.scalar.activation(
                    out=xT8[:, dt, ds(c0, n)], in_=src, func=AF.Identity,
                    bias=gb1_sb[:, dt : dt + 1], scale=g1_sb[:, dt : dt + 1],
                )
                nc.scalar.activation(
                    out=xT64[:, dt, ds(c0, n)], in_=src, func=AF.Identity,
                    bias=gb164_sb[:, dt : dt + 1], scale=g164_sb[:, dt : dt + 1],
                )

            _emit_ln_T(nc, P, residT[sb][:, :, ds(loc, n)], sq, stats, write_x, n=n)

        def emit_ffn(c0, n):
            hT = hT_p.tile([128, DT, n], FP8, tag="hT")
            for ht in range(DT):
                ps = ps_proj.tile([128, 512], FP32, tag="proj_ps", name="ffn1_ps")[:, :n]
                for kp in range(0, DT, 2):
                    nc.tensor.matmul(
                        ps, lhsT=late["w1"][:, kp : kp + 2, ts(ht, 128)],
                        rhs=xT8[:, kp : kp + 2, ds(c0, n)],
                        start=(kp == 0), stop=(kp == DT - 2), perf_mode=DR,
                    )
                nc.scalar.activation(
                    out=hT[:, ht, :], in_=ps, func=AF.Relu,
                    bias=late["b1"][:, ht : ht + 1], scale=1.0 / FFN_SCALE,
                )
            # resid2 holds 64*(x + ffn2(x)); LN2 normalizes the scale away
            resid2 = resid_p.tile([128, DT, n], MM_DT, tag="resid2")
            for dt in range(DT):
                ps = ps_proj.tile([128, 512], FP32, tag="proj_ps", name="ffn2_ps")[:, :n]
                for hp in range(0, DT, 2):
                    nc.tensor.matmul(
                        ps, lhsT=late["w2"][:, hp : hp + 2, ts(dt, 128)],
                        rhs=hT[:, hp : hp + 2, :],
                        start=(hp == 0), stop=(hp == DT - 2), perf_mode=DR,
                    )
                nc.vector.scalar_tensor_tensor(
                    out=resid2[:, dt, :], in0=ps, scalar=late["b2"][:, dt : dt + 1],
                    in1=xT64[:, dt, ds(c0, n)], op0=OP.add, op1=OP.add,
                )
            sq2 = hT_p.tile([128, DT, n], MM_DT, tag="sq2")
            stats2 = small.tile([1, 1024], MM_DT, tag="stats2")
            out_sb = out_p.tile([128, DT, n], FP32, tag="out_sb")

            def write_out(dt, src, out_sb=out_sb):
                nc.scalar.activation(
                    out=out_sb[:, dt, :], in_=src, func=AF.Identity,
                    bias=late["gb2"][:, dt : dt + 1], scale=late["g2"][:, dt : dt + 1],
                )
                nc.sync.dma_start(out=outT_t[:, dt, ds(c0, n)], in_=out_sb[:, dt, :])

            _emit_ln_T(nc, P, resid2[:], sq2, stats2, write_out, n=n)

        prev = None
        for b in range(NBLK):
            acc = emit_front(b)
            if b == 0:
                proj_T(wq_sb, bq_sb, q_ins[1], qTp, 512, 512)
                load_late_consts()
            if prev is not None:
                emit_mid(b - 1, prev)
            if b == 4:
                emit_ln1(0, 512)
            prev = acc
        emit_mid(NBLK - 1, prev)
        # FFN(0) is emitted BEFORE LN1(1): its matmuls are ready immediately,
        # so the PE works through them while mid(7)'s DVE adds drain, instead
        # of stalling behind LN1(1)'s resid-gated stats matmuls. The last
        # superblock runs as two 256-token chunks so the final LN2 is short.
        emit_ffn(0, 512)
        emit_ln1(512, 512)
        emit_ffn(512, 256)
        emit_ffn(768, 256)

    nc.finalize()
    return nc


def kernel(**inputs):
    if "prog" not in _CACHE:
        _CACHE["prog"] = build_program()
    nc = _CACHE["prog"]

    import ml_dtypes

    f32 = lambda x: np.ascontiguousarray(np.asarray(x), dtype=np.float32)
    bf16 = lambda x: np.ascontiguousarray(np.asarray(x, dtype=np.float32).astype(ml_dtypes.bfloat16))
    fp8 = lambda x, s=1.0: np.ascontiguousarray(
        (np.asarray(x, dtype=np.float32) * s).astype(ml_dtypes.float8_e4m3)
    )
    query, key_, value = f32(inputs["query"]), f32(inputs["key"]), f32(inputs["value"])

    shared = {
        n: f32(inputs[n])
        for n in ("b_q", "b_k", "b_v", "ffn_b1",
                  "ln1_g", "ln1_b", "ln2_g", "ln2_b")
    }
    shared["ffn_b2"] = f32(inputs["ffn_b2"]) * FFN_SCALE
    shared["ln1_g64"] = f32(inputs["ln1_g"]) * FFN_SCALE
    shared["ln1_b64"] = f32(inputs["ln1_b"]) * FFN_SCALE
    shared["w_v"] = bf16(inputs["w_v"])
    shared["w_q"] = fp8(inputs["w_q"], WQK_SCALE)
    shared["w_k"] = fp8(inputs["w_k"], WQK_SCALE)
    shared["ffn_w1"] = fp8(inputs["ffn_w1"], FFN_SCALE)
    shared["ffn_w2"] = fp8(inputs["ffn_w2"], FFN_SCALE)
    p_idx = np.arange(128)[:, None]
    k_idx = np.arange(512)[None, :]
    shared["cmask"] = ((k_idx - 4 * p_idx >= 0) & (k_idx - 4 * p_idx <= 3)).astype(np.float32)
    shared["cident"] = np.eye(128, dtype=np.float32)

    in_maps = []
    for c in range(NCORES):
        bi, half = c // 2, c % 2
        w0 = half * WPC
        m = dict(shared)
        m["qT"] = fp8(query[bi, w0 : w0 + WPC, :].T)
        m["kT"] = fp8(key_[bi, w0 * F : (w0 + WPC) * F, :].T)
        m["vT"] = bf16(value[bi, w0 * F : (w0 + WPC) * F, :].T)
        in_maps.append(m)

    res = run_bass_kernel_spmd(nc, in_maps, core_ids=list(range(NCORES)))
    _CACHE["last_result"] = res
    out = np.empty((B, SQ, D), dtype=np.float32)
    for c in range(NCORES):
        bi, half = c // 2, c % 2
        w0 = half * WPC
        out[bi, w0 : w0 + WPC, :] = res.results[c]["outT"].T
    return out



# revision 46
# speedup vs baseline: 1.0551x; 1.0551x over previous
"""Trainium2 Bass kernel for nn_AttentionSampling (sparse window attention block).

Sharding: 8 cores, data-parallel, 1024 windows (half a batch) per core; windows are
independent so there is no cross-core communication. Activations live in a transposed
[d, tokens] layout (host pre-transposes q/k) so projections run weight-stationary.

Precision: q/k/ffn projections run fp8e4 DoubleRow (weights host-prescaled x16/x64,
folded back via ACT scale; ffn2's 1/64 rides the residual — LN2 is scale-invariant).
Scores, value path and LN broadcasts are bf16; LN stats accumulate in fp32 PSUM.

Value path (exact algebra): the windowed weighted-sum commutes with the value
projection, so the kernel downsamples RAW value first — masked scores are
transposed (PE) into the banded [keys, windows] operand; each (key-chunk, d-tile)
pair is a single 32-col matmul since chunk c only feeds windows 32c..32c+31 —
then projects the [512, 128]-shrunk vs through w_v (4 matmuls vs 16).

Pipeline: block b's value path (PE work gated on DVE mask + ACT copies) is
emitted after block b+1's k-proj/scores so the PE never stalls on them;
residual-transpose + LN trail two blocks behind.
"""

import sys
import types

# If BASS_TRACE is set in an environment whose antenv package lacks
# axon_hooks, run_bass_kernel_spmd would crash on import; provide a stub
# (a None hook makes bass_utils skip tracing gracefully).
try:
    import antenv.axon_hooks  # noqa: F401
except ImportError:
    _m = types.ModuleType("antenv.axon_hooks")
    _m.get_axon_ntff_profile_hook = lambda: None
    _m.set_axon_ntff_profile_hook = lambda h: None
    sys.modules["antenv.axon_hooks"] = _m
    try:
        import antenv

        antenv.axon_hooks = _m
    except ImportError:
        pass

import contextlib

import numpy as np

import concourse.bass as bass
import concourse.bacc as bacc_mod
import concourse.mybir as mybir
import concourse.tile as tile
from concourse.bass import ts, ds
from concourse.bass_utils import run_bass_kernel_spmd

FP32 = mybir.dt.float32
FP32R = mybir.dt.float32r
FP8 = mybir.dt.float8e4
AF = mybir.ActivationFunctionType
OP = mybir.AluOpType
DR = mybir.MatmulPerfMode.DoubleRow

MM_DT = mybir.dt.bfloat16  # matmul operands; attention weights/LN stay fp32
# fp8 weight pre-scales (host multiplies weights up so fp8 stays in normal
# range; the ACT after each matmul folds the inverse back in)
WQK_SCALE = 16.0
FFN_SCALE = 64.0

B, SQ, SK, D, F = 4, 2048, 8192, 512, 4
NCORES = 8
WPC = B * SQ // NCORES        # 1024 windows (= tokens) per core
KPC = WPC * F                 # 4096 keys per core
NBLK = WPC // 128             # 8 attention blocks: 128 windows / 512 keys
NSB = WPC // 512              # 2 superblocks of 512 tokens
DT = D // 128                 # 4 d-tiles
EPS = 1e-5

_CACHE = {}


def _emit_ln_T(nc, P, resid_view, sq_tile, stats_sb, out_cb, n=512):
    """Transposed LayerNorm over D for an n-token chunk.

    resid_view/sq_tile: [128, DT, n]; sq_tile doubles as apply scratch.
    stats_sb: [1, 1024] (mean at 0, rstd at 512, each n long).
    out_cb(dt, src): write normalized+affine output for d-tile dt from src.
    """
    mean = stats_sb[:, :n]          # bf16 (bc matmul rhs streams 1 cyc/row)
    rstd_bf = stats_sb[:, 512 : 512 + n]

    nc.vector.tensor_tensor(sq_tile[:], resid_view, resid_view, op=OP.mult)

    # resid/sq tiles are bf16 so the stats matmuls stream at 1 cyc/row
    st_sum = P["st"].tile([1, 512], FP32, tag="st", name="st_sum")[:, :n]
    for dt in range(DT):
        nc.tensor.matmul(
            st_sum, lhsT=P["ones_col"], rhs=resid_view[:, dt, :],
            start=(dt == 0), stop=(dt == DT - 1),
        )
    nc.scalar.activation(out=mean, in_=st_sum, func=AF.Copy, scale=1.0 / D)

    st_sq = P["st"].tile([1, 512], FP32, tag="st", name="st_sq")[:, :n]
    for dt in range(DT):
        nc.tensor.matmul(
            st_sq, lhsT=P["ones_col"], rhs=sq_tile[:, dt, :],
            start=(dt == 0), stop=(dt == DT - 1),
        )
    e2 = P["small"].tile([1, 512], FP32, tag="e2", name="e2")[:, :n]
    var = P["small"].tile([1, 512], FP32, tag="var", name="var")[:, :n]
    nc.scalar.activation(out=e2, in_=st_sq, func=AF.Copy, scale=1.0 / D)
    nc.vector.tensor_tensor(var, mean, mean, op=OP.mult)
    nc.vector.tensor_tensor(var, e2, var, op=OP.subtract)
    nc.scalar.activation(out=var, in_=var, func=AF.Sqrt, bias=P["eps_t"], scale=1.0)
    rstd32 = P["small"].tile([1, 512], FP32, tag="rstd32", name="rstd32")[:, :n]
    nc.vector.reciprocal_approx_fast(out=rstd32, in_=var)
    nc.vector.tensor_scalar_add(rstd_bf, rstd32, 0.0)

    # round 1: subtract broadcast mean (sq_tile becomes the scratch)
    bc = P["bc"].tile([128, 512], FP32, tag="bc", name="bc_mean")[:, :n]
    nc.tensor.matmul(bc, lhsT=P["ones_row"], rhs=mean, start=True, stop=True)
    for dt in range(DT):
        nc.vector.tensor_tensor(sq_tile[:, dt, :], resid_view[:, dt, :], bc, op=OP.subtract)
    # round 2: multiply broadcast rstd (in place), then affine via ACT
    bc2 = P["bc"].tile([128, 512], FP32, tag="bc", name="bc_rstd")[:, :n]
    nc.tensor.matmul(bc2, lhsT=P["ones_row"], rhs=rstd_bf, start=True, stop=True)
    for dt in range(DT):
        nc.vector.tensor_tensor(sq_tile[:, dt, :], sq_tile[:, dt, :], bc2, op=OP.mult)
        out_cb(dt, sq_tile[:, dt, :])


def build_program():
    nc = bacc_mod.Bacc(None, target_bir_lowering=False)

    qT_d = nc.dram_tensor("qT", [D, WPC], FP8, kind="ExternalInput")
    kT_d = nc.dram_tensor("kT", [D, KPC], FP8, kind="ExternalInput")
    vT_d = nc.dram_tensor("vT", [D, KPC], MM_DT, kind="ExternalInput")
    wq_d = nc.dram_tensor("w_q", [D, D], FP8, kind="ExternalInput")
    wk_d = nc.dram_tensor("w_k", [D, D], FP8, kind="ExternalInput")
    wv_d = nc.dram_tensor("w_v", [D, D], MM_DT, kind="ExternalInput")
    w1_d = nc.dram_tensor("ffn_w1", [D, D], FP8, kind="ExternalInput")
    w2_d = nc.dram_tensor("ffn_w2", [D, D], FP8, kind="ExternalInput")
    bq_d = nc.dram_tensor("b_q", [D], FP32, kind="ExternalInput")
    bk_d = nc.dram_tensor("b_k", [D], FP32, kind="ExternalInput")
    bv_d = nc.dram_tensor("b_v", [D], FP32, kind="ExternalInput")
    b1_d = nc.dram_tensor("ffn_b1", [D], FP32, kind="ExternalInput")
    b2_d = nc.dram_tensor("ffn_b2", [D], FP32, kind="ExternalInput")
    g1_d = nc.dram_tensor("ln1_g", [D], FP32, kind="ExternalInput")
    gb1_d = nc.dram_tensor("ln1_b", [D], FP32, kind="ExternalInput")
    g164_d = nc.dram_tensor("ln1_g64", [D], FP32, kind="ExternalInput")
    gb164_d = nc.dram_tensor("ln1_b64", [D], FP32, kind="ExternalInput")
    g2_d = nc.dram_tensor("ln2_g", [D], FP32, kind="ExternalInput")
    gb2_d = nc.dram_tensor("ln2_b", [D], FP32, kind="ExternalInput")
    mask_d = nc.dram_tensor("cmask", [128, 512], FP32, kind="ExternalInput")
    ident_d = nc.dram_tensor("cident", [128, 128], FP32, kind="ExternalInput")
    outT_d = nc.dram_tensor("outT", [D, WPC], FP32, kind="ExternalOutput")

    qT_t = qT_d.rearrange("(o p) n -> p o n", p=128)
    kT_t = kT_d.rearrange("(o p) n -> p o n", p=128)
    vT_t = vT_d.rearrange("(o p) n -> p o n", p=128)
    outT_t = outT_d.rearrange("(o p) n -> p o n", p=128)

    with tile.TileContext(nc) as tc, contextlib.ExitStack() as ctx:
        singles = ctx.enter_context(tc.tile_pool(name="singles", bufs=1))
        inp = ctx.enter_context(tc.tile_pool(name="inp", bufs=4))
        ktp_p = ctx.enter_context(tc.tile_pool(name="ktp", bufs=1))
        att_p = ctx.enter_context(tc.tile_pool(name="att", bufs=2))
        resid_p = ctx.enter_context(tc.tile_pool(name="resid", bufs=2))
        hT_p = ctx.enter_context(tc.tile_pool(name="hT", bufs=1))
        out_p = ctx.enter_context(tc.tile_pool(name="outp", bufs=2))
        small = ctx.enter_context(tc.tile_pool(name="small", bufs=1))
        ps_proj = ctx.enter_context(tc.tile_pool(name="ps_proj", bufs=3, space="PSUM"))
        ps_vf = ctx.enter_context(tc.tile_pool(name="ps_vf", bufs=2, space="PSUM"))
        ps_tr = ctx.enter_context(tc.tile_pool(name="ps_tr", bufs=1, space="PSUM"))
        ps_st = ctx.enter_context(tc.tile_pool(name="ps_st", bufs=1, space="PSUM"))
        ps_bc = ctx.enter_context(tc.tile_pool(name="ps_bc", bufs=1, space="PSUM"))

        def load_w(d, tg, dt_=MM_DT):
            t = singles.tile([128, DT, 512], dt_, tag=tg)
            nc.sync.dma_start(out=t, in_=d.rearrange("(o p) n -> p o n", p=128))
            return t

        def load_b(d, tg):
            t = singles.tile([128, DT], FP32, tag=tg)
            nc.sync.dma_start(out=t, in_=d.rearrange("(o p) -> p o", p=128))
            return t

        # issue order matters: Sync issues DMAs in program order, and the PE's
        # first work (q-proj then k-proj of block 0) must not wait behind a
        # dozen constant loads.
        # DMA issue order == PE consumption order: wq+q(sb0) gate the very
        # first matmul, then k0 for k-proj(0); the 2MB v0 comes later.
        wq_sb = load_w(wq_d, "wq", FP8)
        bq_sb = load_b(bq_d, "bq")
        q_ins = []
        q_in = inp.tile([128, DT, 512], FP8, tag="in_t", name="q_in0")
        nc.sync.dma_start(out=q_in, in_=qT_t[:, :, ts(0, 512)])
        q_ins.append(q_in)
        k0 = inp.tile([128, DT, 512], FP8, tag="in_t", name="k0")
        nc.sync.dma_start(out=k0, in_=kT_t[:, :, ts(0, 512)])

        wk_sb = load_w(wk_d, "wk", FP8)
        bk_sb = load_b(bk_d, "bk")
        q_in = inp.tile([128, DT, 512], FP8, tag="in_t", name="q_in1")
        nc.sync.dma_start(out=q_in, in_=qT_t[:, :, ts(1, 512)])
        q_ins.append(q_in)
        v0 = inp.tile([128, DT, 512], MM_DT, tag="in_v", bufs=2, name="v0")
        nc.sync.dma_start(out=v0, in_=vT_t[:, :, ts(0, 512)])
        kv0 = [k0, v0]
        wv_sb = load_w(wv_d, "wv")
        mask = singles.tile([128, 512], FP32, tag="mask")
        nc.sync.dma_start(out=mask, in_=mask_d[:, :])
        bv_rep = singles.tile([128, 512], FP32, tag="bv_rep")
        nc.gpsimd.dma_start(
            out=bv_rep, in_=bass.AP(tensor=bv_d, offset=0, ap=[[0, 128], [1, 512]])
        )
        identity = singles.tile([128, 128], FP32, tag="ident")
        nc.sync.dma_start(out=identity, in_=ident_d[:, :])
        g1_sb = load_b(g1_d, "g1")
        gb1_sb = load_b(gb1_d, "gb1")
        g164_sb = load_b(g164_d, "g164")
        gb164_sb = load_b(gb164_d, "gb164")
        ones_col = singles.tile([128, 1], MM_DT, tag="ones_col")
        nc.gpsimd.memset(ones_col, 1.0)
        ones_row = singles.tile([1, 128], MM_DT, tag="ones_row")
        nc.gpsimd.memset(ones_row, 1.0)
        eps_t = singles.tile([1, 1], FP32, tag="eps")
        nc.gpsimd.memset(eps_t, EPS)
        late = {}

        def load_late_consts():
            late["w1"] = load_w(w1_d, "w1", FP8)
            late["b1"] = load_b(b1_d, "b1")
            late["w2"] = load_w(w2_d, "w2", FP8)
            late["b2"] = load_b(b2_d, "b2")  # host pre-scales by FFN_SCALE
            late["g2"] = load_b(g2_d, "g2")
            late["gb2"] = load_b(gb2_d, "gb2")

        P = {
            "st": ps_st, "bc": ps_bc, "small": small,
            "ones_col": ones_col, "ones_row": ones_row, "eps_t": eps_t,
        }

        qTp = singles.tile([128, DT, WPC], MM_DT, tag="qTp")
        xT8 = singles.tile([128, DT, WPC], FP8, tag="xT8")
        xT64 = singles.tile([128, DT, WPC], MM_DT, tag="xT64")

        def proj_T(w_sb, bias_sb, in_sb, out_sb, out_col0, n):
            # fp8 DoubleRow: two 128-row k-tiles per matmul at 0.5 cyc/row
            for do in range(DT):
                ps = ps_proj.tile([128, 512], FP32, tag="proj_ps", name="proj_ps")
                ps = ps[:, :n]
                for kp in range(0, DT, 2):
                    nc.tensor.matmul(
                        ps, lhsT=w_sb[:, kp : kp + 2, ts(do, 128)],
                        rhs=in_sb[:, kp : kp + 2, :n],
                        start=(kp == 0), stop=(kp == DT - 2), perf_mode=DR,
                    )
                nc.scalar.activation(
                    out=out_sb[:, do, ds(out_col0, n)], in_=ps, func=AF.Relu,
                    bias=bias_sb[:, do : do + 1], scale=1.0 / WQK_SCALE,
                )

        # ---- phase 1: q projection of superblock 0 only; sb1 slots in
        # after k-proj(0) starts so the PE never waits on the q1 DMA ----
        proj_T(wq_sb, bq_sb, q_ins[0], qTp, 0, 512)

        # ---- phase 2: attention ----
        residT = {}  # superblock -> tile [128, DT, 512]

        def emit_front(b):
            if b == 0:
                k_in, v_in = kv0
            else:
                k_in = inp.tile([128, DT, 512], FP8, tag="in_t")
                nc.sync.dma_start(out=k_in, in_=kT_t[:, :, ts(b, 512)])
                v_in = inp.tile([128, DT, 512], MM_DT, tag="in_v", bufs=2)
                nc.sync.dma_start(out=v_in, in_=vT_t[:, :, ts(b, 512)])

            kTp = ktp_p.tile([128, DT, 512], MM_DT, tag="kTp")
            proj_T(wk_sb, bk_sb, k_in, kTp, 0, 512)

            sc_ps = ps_proj.tile([128, 512], FP32, tag="proj_ps", name="sc_ps")
            for ki in range(DT):
                nc.tensor.matmul(
                    sc_ps, lhsT=qTp[:, ki, ts(b, 128)], rhs=kTp[:, ki, :],
                    start=(ki == 0), stop=(ki == DT - 1),
                )
            sm = att_p.tile([128, 512], FP32, tag="sm")
            nc.vector.tensor_tensor(sm, sc_ps, mask, op=OP.mult)
            wts = small.tile([128, F], FP32, tag="wts")
            nc.vector.tensor_reduce(
                out=wts, in_=sm.rearrange("p (kw f) -> p f kw", f=F),
                axis=mybir.AxisListType.X, op=OP.add,
            )
            # v projection (f-strided) + fused weighted accumulation from PSUM
            acc = bv_rep
            for f in range(F):
                ps = ps_vf.tile([128, 512], FP32, tag="vf", name="vf_ps")
                for ki in range(DT):
                    nc.tensor.matmul(
                        ps, lhsT=v_in[:, ki, f::4], rhs=wv_sb[:, ki, :],
                        start=(ki == 0), stop=(ki == DT - 1),
                    )
                tg = "ao_final" if f == F - 1 else "ao_acc"
                nxt = att_p.tile([128, 512], FP32, tag=tg, name="ao_acc")
                nc.vector.scalar_tensor_tensor(
                    out=nxt, in0=ps, scalar=wts[:, f : f + 1], in1=acc,
                    op0=OP.mult, op1=OP.add,
                )
                acc = nxt
            return acc

        def emit_mid(b, acc):
            sb, col = b // 4, (b % 4) * 128
            if col == 0:
                residT[sb] = resid_p.tile([128, DT, 512], MM_DT, tag="residT", name="residT")
            r = residT[sb]
            for dt in range(DT):
                ps_t = ps_tr.tile([128, 128], FP32, tag="tr_ps", name="tr_ps")
                nc.tensor.transpose(ps_t, acc[:, ts(dt, 128)], identity)
                nc.vector.tensor_tensor(
                    r[:, dt, ds(col, 128)], ps_t, qTp[:, dt, ts(b, 128)], op=OP.add
                )

        def emit_ln1(c0, n):
            sb, loc = c0 // 512, c0 % 512
            sq = resid_p.tile([128, DT, n], MM_DT, tag="sq1")
            stats = small.tile([1, 1024], MM_DT, tag="stats1")

            def write_x(dt, src):
                # dual write: fp8 x for the ffn1 DoubleRow rhs, and bf16
                # 64*x (FFN_SCALE) so the ffn2 residual stt needs no
                # unscale op — LN2 is invariant to the uniform 64x.
                nc.scalar.activation(
                    out=xT8[:, dt, ds(c0, n)], in_=src, func=AF.Identity,
                    bias=gb1_sb[:, dt : dt + 1], scale=g1_sb[:, dt : dt + 1],
                )
                nc.scalar.activation(
                    out=xT64[:, dt, ds(c0, n)], in_=src, func=AF.Identity,
                    bias=gb164_sb[:, dt : dt + 1], scale=g164_sb[:, dt : dt + 1],
                )

            _emit_ln_T(nc, P, residT[sb][:, :, ds(loc, n)], sq, stats, write_x, n=n)

        def emit_ffn(c0, n):
            hT = hT_p.tile([128, DT, n], FP8, tag="hT")
            for ht in range(DT):
                ps = ps_proj.tile([128, 512], FP32, tag="proj_ps", name="ffn1_ps")[:, :n]
                for kp in range(0, DT, 2):
                    nc.tensor.matmul(
                        ps, lhsT=late["w1"][:, kp : kp + 2, ts(ht, 128)],
                        rhs=xT8[:, kp : kp + 2, ds(c0, n)],
                        start=(kp == 0), stop=(kp == DT - 2), perf_mode=DR,
                    )
                nc.scalar.activation(
                    out=hT[:, ht, :], in_=ps, func=AF.Relu,
                    bias=late["b1"][:, ht : ht + 1], scale=1.0 / FFN_SCALE,
                )
            # resid2 holds 64*(x + ffn2(x)); LN2 normalizes the scale away
            resid2 = resid_p.tile([128, DT, n], MM_DT, tag="resid2")
            for dt in range(DT):
                ps = ps_proj.tile([128, 512], FP32, tag="proj_ps", name="ffn2_ps")[:, :n]
                for hp in range(0, DT, 2):
                    nc.tensor.matmul(
                        ps, lhsT=late["w2"][:, hp : hp + 2, ts(dt, 128)],
                        rhs=hT[:, hp : hp + 2, :],
                        start=(hp == 0), stop=(hp == DT - 2), perf_mode=DR,
                    )
                nc.vector.scalar_tensor_tensor(
                    out=resid2[:, dt, :], in0=ps, scalar=late["b2"][:, dt : dt + 1],
                    in1=xT64[:, dt, ds(c0, n)], op0=OP.add, op1=OP.add,
                )
            sq2 = hT_p.tile([128, DT, n], MM_DT, tag="sq2")
            stats2 = small.tile([1, 1024], MM_DT, tag="stats2")
            out_sb = out_p.tile([128, DT, n], FP32, tag="out_sb")

            def write_out(dt, src, out_sb=out_sb):
                nc.scalar.activation(
                    out=out_sb[:, dt, :], in_=src, func=AF.Identity,
                    bias=late["gb2"][:, dt : dt + 1], scale=late["g2"][:, dt : dt + 1],
                )
                nc.sync.dma_start(out=outT_t[:, dt, ds(c0, n)], in_=out_sb[:, dt, :])

            _emit_ln_T(nc, P, resid2[:], sq2, stats2, write_out, n=n)

        prev = None
        for b in range(NBLK):
            acc = emit_front(b)
            if b == 0:
                proj_T(wq_sb, bq_sb, q_ins[1], qTp, 512, 512)
                load_late_consts()
            if prev is not None:
                emit_mid(b - 1, prev)
            if b == 4:
                emit_ln1(0, 512)
            prev = acc
        emit_mid(NBLK - 1, prev)
        emit_ln1(512, 512)
        # FFN(0) fills the PE while LN1(1)'s DVE/ACT chain drains; the last
        # superblock runs as two 256-token chunks so the final LN2 is short.
        emit_ffn(0, 512)
        emit_ffn(512, 256)
        emit_ffn(768, 256)

    nc.finalize()
    return nc


def kernel(**inputs):
    if "prog" not in _CACHE:
        _CACHE["prog"] = build_program()
    nc = _CACHE["prog"]

    import ml_dtypes

    f32 = lambda x: np.ascontiguousarray(np.asarray(x), dtype=np.float32)
    bf16 = lambda x: np.ascontiguousarray(np.asarray(x, dtype=np.float32).astype(ml_dtypes.bfloat16))
    fp8 = lambda x, s=1.0: np.ascontiguousarray(
        (np.asarray(x, dtype=np.float32) * s).astype(ml_dtypes.float8_e4m3)
    )
    query, key_, value = f32(inputs["query"]), f32(inputs["key"]), f32(inputs["value"])

    shared = {
        n: f32(inputs[n])
        for n in ("b_q", "b_k", "b_v", "ffn_b1",
                  "ln1_g", "ln1_b", "ln2_g", "ln2_b")
    }
    shared["ffn_b2"] = f32(inputs["ffn_b2"]) * FFN_SCALE
    shared["ln1_g64"] = f32(inputs["ln1_g"]) * FFN_SCALE
    shared["ln1_b64"] = f32(inputs["ln1_b"]) * FFN_SCALE
    shared["w_v"] = bf16(inputs["w_v"])
    shared["w_q"] = fp8(inputs["w_q"], WQK_SCALE)
    shared["w_k"] = fp8(inputs["w_k"], WQK_SCALE)
    shared["ffn_w1"] = fp8(inputs["ffn_w1"], FFN_SCALE)
    shared["ffn_w2"] = fp8(inputs["ffn_w2"], FFN_SCALE)
    p_idx = np.arange(128)[:, None]
    k_idx = np.arange(512)[None, :]
    shared["cmask"] = ((k_idx - 4 * p_idx >= 0) & (k_idx - 4 * p_idx <= 3)).astype(np.float32)
    shared["cident"] = np.eye(128, dtype=np.float32)

    in_maps = []
    for c in range(NCORES):
        bi, half = c // 2, c % 2
        w0 = half * WPC
        m = dict(shared)
        m["qT"] = fp8(query[bi, w0 : w0 + WPC, :].T)
        m["kT"] = fp8(key_[bi, w0 * F : (w0 + WPC) * F, :].T)
        m["vT"] = bf16(value[bi, w0 * F : (w0 + WPC) * F, :].T)
        in_maps.append(m)

    res = run_bass_kernel_spmd(nc, in_maps, core_ids=list(range(NCORES)))
    _CACHE["last_result"] = res
    out = np.empty((B, SQ, D), dtype=np.float32)
    for c in range(NCORES):
        bi, half = c // 2, c % 2
        w0 = half * WPC
        out[bi, w0 : w0 + WPC, :] = res.results[c]["outT"].T
    return out



# revision 49
# speedup vs baseline: 1.0636x; 1.0081x over previous
"""Trainium2 Bass kernel for nn_AttentionSampling (sparse window attention block).

Sharding: 8 cores, data-parallel, 1024 windows (half a batch) per core; windows are
independent so there is no cross-core communication. Activations live in a transposed
[d, tokens] layout (host pre-transposes q/k) so projections run weight-stationary.

Precision: q/k/ffn projections run fp8e4 DoubleRow (weights host-prescaled x16/x64,
folded back via ACT scale; ffn2's 1/64 rides the residual — LN2 is scale-invariant).
Scores, value path and LN broadcasts are bf16; LN stats accumulate in fp32 PSUM.

Value path (exact algebra): the windowed weighted-sum commutes with the value
projection, so the kernel downsamples RAW value first — masked scores are
transposed (PE) into the banded [keys, windows] operand; each (key-chunk, d-tile)
pair is a single 32-col matmul since chunk c only feeds windows 32c..32c+31 —
then projects the [512, 128]-shrunk vs through w_v (4 matmuls vs 16).

Pipeline: block b's value path (PE work gated on DVE mask + ACT copies) is
emitted after block b+1's k-proj/scores so the PE never stalls on them;
residual-transpose + LN trail two blocks behind.
"""

import sys
import types

# If BASS_TRACE is set in an environment whose antenv package lacks
# axon_hooks, run_bass_kernel_spmd would crash on import; provide a stub
# (a None hook makes bass_utils skip tracing gracefully).
try:
    import antenv.axon_hooks  # noqa: F401
except ImportError:
    _m = types.ModuleType("antenv.axon_hooks")
    _m.get_axon_ntff_profile_hook = lambda: None
    _m.set_axon_ntff_profile_hook = lambda h: None
    sys.modules["antenv.axon_hooks"] = _m
    try:
        import antenv

        antenv.axon_hooks = _m
    except ImportError:
        pass

import contextlib

import numpy as np

import concourse.bass as bass
import concourse.bacc as bacc_mod
import concourse.mybir as mybir
import concourse.tile as tile
from concourse.bass import ts, ds
from concourse.bass_utils import run_bass_kernel_spmd

FP32 = mybir.dt.float32
FP32R = mybir.dt.float32r
FP8 = mybir.dt.float8e4
AF = mybir.ActivationFunctionType
OP = mybir.AluOpType
DR = mybir.MatmulPerfMode.DoubleRow

MM_DT = mybir.dt.bfloat16  # matmul operands; attention weights/LN stay fp32
# fp8 weight pre-scales (host multiplies weights up so fp8 stays in normal
# range; the ACT after each matmul folds the inverse back in)
WQK_SCALE = 16.0
FFN_SCALE = 64.0

B, SQ, SK, D, F = 4, 2048, 8192, 512, 4
NCORES = 8
WPC = B * SQ // NCORES        # 1024 windows (= tokens) per core
KPC = WPC * F                 # 4096 keys per core
NBLK = WPC // 128             # 8 attention blocks: 128 windows / 512 keys
NSB = WPC // 512              # 2 superblocks of 512 tokens
DT = D // 128                 # 4 d-tiles
EPS = 1e-5

_CACHE = {}


def _emit_ln_T(nc, P, resid_view, sq_tile, stats_sb, out_cb, n=512):
    """Transposed LayerNorm over D for an n-token chunk.

    resid_view/sq_tile: [128, DT, n]; sq_tile doubles as apply scratch.
    stats_sb: [1, 1024] (mean at 0, rstd at 512, each n long).
    out_cb(dt, src): write normalized+affine output for d-tile dt from src.
    """
    mean = stats_sb[:, :n]          # bf16 (bc matmul rhs streams 1 cyc/row)
    rstd_bf = stats_sb[:, 512 : 512 + n]

    nc.vector.tensor_tensor(sq_tile[:], resid_view, resid_view, op=OP.mult)

    # resid/sq tiles are bf16 so the stats matmuls stream at 1 cyc/row
    st_sum = P["st"].tile([1, 512], FP32, tag="st", name="st_sum")[:, :n]
    for dt in range(DT):
        nc.tensor.matmul(
            st_sum, lhsT=P["ones_col"], rhs=resid_view[:, dt, :],
            start=(dt == 0), stop=(dt == DT - 1),
        )
    nc.scalar.activation(out=mean, in_=st_sum, func=AF.Copy, scale=1.0 / D)

    st_sq = P["st"].tile([1, 512], FP32, tag="st", name="st_sq")[:, :n]
    for dt in range(DT):
        nc.tensor.matmul(
            st_sq, lhsT=P["ones_col"], rhs=sq_tile[:, dt, :],
            start=(dt == 0), stop=(dt == DT - 1),
        )
    e2 = P["small"].tile([1, 512], FP32, tag="e2", name="e2")[:, :n]
    var = P["small"].tile([1, 512], FP32, tag="var", name="var")[:, :n]
    nc.scalar.activation(out=e2, in_=st_sq, func=AF.Copy, scale=1.0 / D)
    nc.vector.tensor_tensor(var, mean, mean, op=OP.mult)
    nc.vector.tensor_tensor(var, e2, var, op=OP.subtract)
    nc.scalar.activation(out=var, in_=var, func=AF.Sqrt, bias=P["eps_t"], scale=1.0)
    rstd32 = P["small"].tile([1, 512], FP32, tag="rstd32", name="rstd32")[:, :n]
    nc.vector.reciprocal_approx_fast(out=rstd32, in_=var)
    nc.vector.tensor_scalar_add(rstd_bf, rstd32, 0.0)

    # round 1: subtract broadcast mean (sq_tile becomes the scratch)
    bc = P["bc"].tile([128, 512], FP32, tag="bc", name="bc_mean")[:, :n]
    nc.tensor.matmul(bc, lhsT=P["ones_row"], rhs=mean, start=True, stop=True)
    for dt in range(DT):
        nc.vector.tensor_tensor(sq_tile[:, dt, :], resid_view[:, dt, :], bc, op=OP.subtract)
    # round 2: multiply broadcast rstd (in place), then affine via ACT
    bc2 = P["bc"].tile([128, 512], FP32, tag="bc", name="bc_rstd")[:, :n]
    nc.tensor.matmul(bc2, lhsT=P["ones_row"], rhs=rstd_bf, start=True, stop=True)
    for dt in range(DT):
        nc.vector.tensor_tensor(sq_tile[:, dt, :], sq_tile[:, dt, :], bc2, op=OP.mult)
        out_cb(dt, sq_tile[:, dt, :])


def build_program():
    nc = bacc_mod.Bacc(None, target_bir_lowering=False)

    qT_d = nc.dram_tensor("qT", [D, WPC], FP8, kind="ExternalInput")
    kT_d = nc.dram_tensor("kT", [D, KPC], FP8, kind="ExternalInput")
    vT_d = nc.dram_tensor("vT", [D, KPC], MM_DT, kind="ExternalInput")
    wq_d = nc.dram_tensor("w_q", [D, D], FP8, kind="ExternalInput")
    wk_d = nc.dram_tensor("w_k", [D, D], FP8, kind="ExternalInput")
    wv_d = nc.dram_tensor("w_v", [D, D], MM_DT, kind="ExternalInput")
    w1_d = nc.dram_tensor("ffn_w1", [D, D], FP8, kind="ExternalInput")
    w2_d = nc.dram_tensor("ffn_w2", [D, D], FP8, kind="ExternalInput")
    bq_d = nc.dram_tensor("b_q", [D], FP32, kind="ExternalInput")
    bk_d = nc.dram_tensor("b_k", [D], FP32, kind="ExternalInput")
    bv_d = nc.dram_tensor("b_v", [D], FP32, kind="ExternalInput")
    b1_d = nc.dram_tensor("ffn_b1", [D], FP32, kind="ExternalInput")
    b2_d = nc.dram_tensor("ffn_b2", [D], FP32, kind="ExternalInput")
    g1_d = nc.dram_tensor("ln1_g", [D], FP32, kind="ExternalInput")
    gb1_d = nc.dram_tensor("ln1_b", [D], FP32, kind="ExternalInput")
    g164_d = nc.dram_tensor("ln1_g64", [D], FP32, kind="ExternalInput")
    gb164_d = nc.dram_tensor("ln1_b64", [D], FP32, kind="ExternalInput")
    g2_d = nc.dram_tensor("ln2_g", [D], FP32, kind="ExternalInput")
    gb2_d = nc.dram_tensor("ln2_b", [D], FP32, kind="ExternalInput")
    mask_d = nc.dram_tensor("cmask", [128, 512], FP32, kind="ExternalInput")
    ident_d = nc.dram_tensor("cident", [128, 128], FP32, kind="ExternalInput")
    outT_d = nc.dram_tensor("outT", [D, WPC], FP32, kind="ExternalOutput")

    qT_t = qT_d.rearrange("(o p) n -> p o n", p=128)
    kT_t = kT_d.rearrange("(o p) n -> p o n", p=128)
    vT_t = vT_d.rearrange("(o p) n -> p o n", p=128)
    outT_t = outT_d.rearrange("(o p) n -> p o n", p=128)

    with tile.TileContext(nc) as tc, contextlib.ExitStack() as ctx:
        singles = ctx.enter_context(tc.tile_pool(name="singles", bufs=1))
        inp = ctx.enter_context(tc.tile_pool(name="inp", bufs=4))
        ktp_p = ctx.enter_context(tc.tile_pool(name="ktp", bufs=1))
        att_p = ctx.enter_context(tc.tile_pool(name="att", bufs=2))
        resid_p = ctx.enter_context(tc.tile_pool(name="resid", bufs=2))
        hT_p = ctx.enter_context(tc.tile_pool(name="hT", bufs=1))
        out_p = ctx.enter_context(tc.tile_pool(name="outp", bufs=2))
        small = ctx.enter_context(tc.tile_pool(name="small", bufs=1))
        ps_proj = ctx.enter_context(tc.tile_pool(name="ps_proj", bufs=3, space="PSUM"))
        ps_vf = ctx.enter_context(tc.tile_pool(name="ps_vf", bufs=2, space="PSUM"))
        ps_tr = ctx.enter_context(tc.tile_pool(name="ps_tr", bufs=1, space="PSUM"))
        ps_st = ctx.enter_context(tc.tile_pool(name="ps_st", bufs=1, space="PSUM"))
        ps_bc = ctx.enter_context(tc.tile_pool(name="ps_bc", bufs=1, space="PSUM"))

        def load_w(d, tg, dt_=MM_DT):
            t = singles.tile([128, DT, 512], dt_, tag=tg)
            nc.sync.dma_start(out=t, in_=d.rearrange("(o p) n -> p o n", p=128))
            return t

        def load_b(d, tg):
            t = singles.tile([128, DT], FP32, tag=tg)
            nc.sync.dma_start(out=t, in_=d.rearrange("(o p) -> p o", p=128))
            return t

        # issue order matters: Sync issues DMAs in program order, and the PE's
        # first work (q-proj then k-proj of block 0) must not wait behind a
        # dozen constant loads.
        # DMA issue order == PE consumption order: wq+q(sb0) gate the very
        # first matmul, then k0 for k-proj(0); the 2MB v0 comes later.
        wq_sb = load_w(wq_d, "wq", FP8)
        bq_sb = load_b(bq_d, "bq")
        q_ins = []
        q_in = inp.tile([128, DT, 512], FP8, tag="in_t", name="q_in0")
        nc.sync.dma_start(out=q_in, in_=qT_t[:, :, ts(0, 512)])
        q_ins.append(q_in)
        k0 = inp.tile([128, DT, 512], FP8, tag="in_t", name="k0")
        nc.sync.dma_start(out=k0, in_=kT_t[:, :, ts(0, 512)])

        wk_sb = load_w(wk_d, "wk", FP8)
        bk_sb = load_b(bk_d, "bk")
        q_in = inp.tile([128, DT, 512], FP8, tag="in_t", name="q_in1")
        nc.sync.dma_start(out=q_in, in_=qT_t[:, :, ts(1, 512)])
        q_ins.append(q_in)
        v0 = inp.tile([128, DT, 512], MM_DT, tag="in_v", bufs=2, name="v0")
        nc.sync.dma_start(out=v0, in_=vT_t[:, :, ts(0, 512)])
        kv0 = [k0, v0]
        wv_sb = load_w(wv_d, "wv")
        mask = singles.tile([128, 512], FP32, tag="mask")
        nc.sync.dma_start(out=mask, in_=mask_d[:, :])
        bv_rep = singles.tile([128, 512], FP32, tag="bv_rep")
        nc.gpsimd.dma_start(
            out=bv_rep, in_=bass.AP(tensor=bv_d, offset=0, ap=[[0, 128], [1, 512]])
        )
        identity = singles.tile([128, 128], FP32, tag="ident")
        nc.sync.dma_start(out=identity, in_=ident_d[:, :])
        g1_sb = load_b(g1_d, "g1")
        gb1_sb = load_b(gb1_d, "gb1")
        g164_sb = load_b(g164_d, "g164")
        gb164_sb = load_b(gb164_d, "gb164")
        ones_col = singles.tile([128, 1], MM_DT, tag="ones_col")
        nc.gpsimd.memset(ones_col, 1.0)
        ones_row = singles.tile([1, 128], MM_DT, tag="ones_row")
        nc.gpsimd.memset(ones_row, 1.0)
        eps_t = singles.tile([1, 1], FP32, tag="eps")
        nc.gpsimd.memset(eps_t, EPS)
        late = {}

        def load_late_consts():
            late["w1"] = load_w(w1_d, "w1", FP8)
            late["b1"] = load_b(b1_d, "b1")
            late["w2"] = load_w(w2_d, "w2", FP8)
            late["b2"] = load_b(b2_d, "b2")  # host pre-scales by FFN_SCALE
            late["g2"] = load_b(g2_d, "g2")
            late["gb2"] = load_b(gb2_d, "gb2")

        P = {
            "st": ps_st, "bc": ps_bc, "small": small,
            "ones_col": ones_col, "ones_row": ones_row, "eps_t": eps_t,
        }

        qTp = singles.tile([128, DT, WPC], MM_DT, tag="qTp")
        xT8 = singles.tile([128, DT, WPC], FP8, tag="xT8")
        xT64 = singles.tile([128, DT, WPC], MM_DT, tag="xT64")

        def proj_T(w_sb, bias_sb, in_sb, out_sb, out_col0, n):
            # fp8 DoubleRow: two 128-row k-tiles per matmul at 0.5 cyc/row
            for do in range(DT):
                ps = ps_proj.tile([128, 512], FP32, tag="proj_ps", name="proj_ps")
                ps = ps[:, :n]
                for kp in range(0, DT, 2):
                    nc.tensor.matmul(
                        ps, lhsT=w_sb[:, kp : kp + 2, ts(do, 128)],
                        rhs=in_sb[:, kp : kp + 2, :n],
                        start=(kp == 0), stop=(kp == DT - 2), perf_mode=DR,
                    )
                nc.scalar.activation(
                    out=out_sb[:, do, ds(out_col0, n)], in_=ps, func=AF.Relu,
                    bias=bias_sb[:, do : do + 1], scale=1.0 / WQK_SCALE,
                )

        # ---- phase 1: q projection of superblock 0 only; sb1 slots in
        # after k-proj(0) starts so the PE never waits on the q1 DMA ----
        proj_T(wq_sb, bq_sb, q_ins[0], qTp, 0, 512)

        # ---- phase 2: attention ----
        residT = {}  # superblock -> tile [128, DT, 512]

        def emit_front(b):
            if b == 0:
                k_in, v_in = kv0
            else:
                k_in = inp.tile([128, DT, 512], FP8, tag="in_t")
                nc.sync.dma_start(out=k_in, in_=kT_t[:, :, ts(b, 512)])
                v_in = inp.tile([128, DT, 512], MM_DT, tag="in_v", bufs=2)
                nc.sync.dma_start(out=v_in, in_=vT_t[:, :, ts(b, 512)])

            kTp = ktp_p.tile([128, DT, 512], MM_DT, tag="kTp")
            proj_T(wk_sb, bk_sb, k_in, kTp, 0, 512)

            sc_ps = ps_proj.tile([128, 512], FP32, tag="proj_ps", name="sc_ps")
            for ki in range(DT):
                nc.tensor.matmul(
                    sc_ps, lhsT=qTp[:, ki, ts(b, 128)], rhs=kTp[:, ki, :],
                    start=(ki == 0), stop=(ki == DT - 1),
                )
            sm = att_p.tile([128, 512], FP32, tag="sm")
            nc.vector.tensor_tensor(sm, sc_ps, mask, op=OP.mult)
            wts = small.tile([128, F], FP32, tag="wts")
            nc.vector.tensor_reduce(
                out=wts, in_=sm.rearrange("p (kw f) -> p f kw", f=F),
                axis=mybir.AxisListType.X, op=OP.add,
            )
            # v projection (f-strided) + fused weighted accumulation from PSUM
            acc = bv_rep
            for f in range(F):
                ps = ps_vf.tile([128, 512], FP32, tag="vf", name="vf_ps")
                for ki in range(DT):
                    nc.tensor.matmul(
                        ps, lhsT=v_in[:, ki, f::4], rhs=wv_sb[:, ki, :],
                        start=(ki == 0), stop=(ki == DT - 1),
                    )
                tg = "ao_final" if f == F - 1 else "ao_acc"
                nxt = att_p.tile([128, 512], FP32, tag=tg, name="ao_acc")
                nc.vector.scalar_tensor_tensor(
                    out=nxt, in0=ps, scalar=wts[:, f : f + 1], in1=acc,
                    op0=OP.mult, op1=OP.add,
                )
                acc = nxt
            return acc

        def emit_mid(b, acc):
            sb, col = b // 4, (b % 4) * 128
            if col == 0:
                residT[sb] = resid_p.tile([128, DT, 512], MM_DT, tag="residT", name="residT")
            r = residT[sb]
            for dt in range(DT):
                ps_t = ps_tr.tile([128, 128], FP32, tag="tr_ps", name="tr_ps")
                nc.tensor.transpose(ps_t, acc[:, ts(dt, 128)], identity)
                nc.vector.tensor_tensor(
                    r[:, dt, ds(col, 128)], ps_t, qTp[:, dt, ts(b, 128)], op=OP.add
                )

        def emit_ln1(c0, n):
            sb, loc = c0 // 512, c0 % 512
            sq = resid_p.tile([128, DT, n], MM_DT, tag="sq1")
            stats = small.tile([1, 1024], MM_DT, tag="stats1")

            def write_x(dt, src):
                # dual write: fp8 x for the ffn1 DoubleRow rhs, and bf16
                # 64*x (FFN_SCALE) so the ffn2 residual stt needs no
                # unscale op — LN2 is invariant to the uniform 64x.
                nc.scalar.activation(
                    out=xT8[:, dt, ds(c0, n)], in_=src, func=AF.Identity,
                    bias=gb1_sb[:, dt : dt + 1], scale=g1_sb[:, dt : dt + 1],
                )
                nc.scalar.activation(
                    out=xT64[:, dt, ds(c0, n)], in_=src, func=AF.Identity,
                    bias=gb164_sb[:, dt : dt + 1], scale=g164_sb[:, dt : dt + 1],
                )

            _emit_ln_T(nc, P, residT[sb][:, :, ds(loc, n)], sq, stats, write_x, n=n)

        def emit_ffn(c0, n):
            hT = hT_p.tile([128, DT, n], FP8, tag="hT")
            for ht in range(DT):
                ps = ps_proj.tile([128, 512], FP32, tag="proj_ps", name="ffn1_ps")[:, :n]
                for kp in range(0, DT, 2):
                    nc.tensor.matmul(
                        ps, lhsT=late["w1"][:, kp : kp + 2, ts(ht, 128)],
                        rhs=xT8[:, kp : kp + 2, ds(c0, n)],
                        start=(kp == 0), stop=(kp == DT - 2), perf_mode=DR,
                    )
                nc.scalar.activation(
                    out=hT[:, ht, :], in_=ps, func=AF.Relu,
                    bias=late["b1"][:, ht : ht + 1], scale=1.0 / FFN_SCALE,
                )
            # resid2 holds 64*(x + ffn2(x)); LN2 normalizes the scale away
            resid2 = resid_p.tile([128, DT, n], MM_DT, tag="resid2")
            for dt in range(DT):
                ps = ps_proj.tile([128, 512], FP32, tag="proj_ps", name="ffn2_ps")[:, :n]
                for hp in range(0, DT, 2):
                    nc.tensor.matmul(
                        ps, lhsT=late["w2"][:, hp : hp + 2, ts(dt, 128)],
                        rhs=hT[:, hp : hp + 2, :],
                        start=(hp == 0), stop=(hp == DT - 2), perf_mode=DR,
                    )
                nc.vector.scalar_tensor_tensor(
                    out=resid2[:, dt, :], in0=ps, scalar=late["b2"][:, dt : dt + 1],
                    in1=xT64[:, dt, ds(c0, n)], op0=OP.add, op1=OP.add,
                )
            sq2 = hT_p.tile([128, DT, n], MM_DT, tag="sq2")
            stats2 = small.tile([1, 1024], MM_DT, tag="stats2")
            out_sb = out_p.tile([128, DT, n], FP32, tag="out_sb")

            def write_out(dt, src, out_sb=out_sb):
                nc.scalar.activation(
                    out=out_sb[:, dt, :], in_=src, func=AF.Identity,
                    bias=late["gb2"][:, dt : dt + 1], scale=late["g2"][:, dt : dt + 1],
                )
                nc.sync.dma_start(out=outT_t[:, dt, ds(c0, n)], in_=out_sb[:, dt, :])

            _emit_ln_T(nc, P, resid2[:], sq2, stats2, write_out, n=n)

        prev = None
        for b in range(NBLK):
            acc = emit_front(b)
            if b == 0:
                proj_T(wq_sb, bq_sb, q_ins[1], qTp, 512, 512)
                load_late_consts()
            if prev is not None:
                emit_mid(b - 1, prev)
            if b == 4:
                emit_ln1(0, 512)
            prev = acc
        emit_mid(NBLK - 1, prev)
        emit_ln1(512, 512)
        # FFN(0) fills the PE while LN1(1)'s DVE/ACT chain drains; the last
        # superblock runs as two 256-token chunks so the final LN2 is short.
        emit_ffn(0, 512)
        emit_ffn(512, 256)
        emit_ffn(768, 256)

    nc.finalize()
    return nc


def kernel(**inputs):
    if "prog" not in _CACHE:
        _CACHE["prog"] = build_program()
    nc = _CACHE["prog"]

    import ml_dtypes

    f32 = lambda x: np.ascontiguousarray(np.asarray(x), dtype=np.float32)
    bf16 = lambda x: np.ascontiguousarray(np.asarray(x, dtype=np.float32).astype(ml_dtypes.bfloat16))
    fp8 = lambda x, s=1.0: np.ascontiguousarray(
        (np.asarray(x, dtype=np.float32) * s).astype(ml_dtypes.float8_e4m3)
    )
    query, key_, value = f32(inputs["query"]), f32(inputs["key"]), f32(inputs["value"])

    shared = {
        n: f32(inputs[n])
        for n in ("b_q", "b_k", "b_v", "ffn_b1",
                  "ln1_g", "ln1_b", "ln2_g", "ln2_b")
    }
    shared["ffn_b2"] = f32(inputs["ffn_b2"]) * FFN_SCALE
    shared["ln1_g64"] = f32(inputs["ln1_g"]) * FFN_SCALE
    shared["ln1_b64"] = f32(inputs["ln1_b"]) * FFN_SCALE
    shared["w_v"] = bf16(inputs["w_v"])
    shared["w_q"] = fp8(inputs["w_q"], WQK_SCALE)
    shared["w_k"] = fp8(inputs["w_k"], WQK_SCALE)
    shared["ffn_w1"] = fp8(inputs["ffn_w1"], FFN_SCALE)
    shared["ffn_w2"] = fp8(inputs["ffn_w2"], FFN_SCALE)
    p_idx = np.arange(128)[:, None]
    k_idx = np.arange(512)[None, :]
    shared["cmask"] = ((k_idx - 4 * p_idx >= 0) & (k_idx - 4 * p_idx <= 3)).astype(np.float32)
    shared["cident"] = np.eye(128, dtype=np.float32)

    in_maps = []
    for c in range(NCORES):
        bi, half = c // 2, c % 2
        w0 = half * WPC
        m = dict(shared)
        m["qT"] = fp8(query[bi, w0 : w0 + WPC, :].T)
        m["kT"] = fp8(key_[bi, w0 * F : (w0 + WPC) * F, :].T)
        m["vT"] = bf16(value[bi, w0 * F : (w0 + WPC) * F, :].T)
        in_maps.append(m)

    res = run_bass_kernel_spmd(nc, in_maps, core_ids=list(range(NCORES)))
    _CACHE["last_result"] = res
    out = np.empty((B, SQ, D), dtype=np.float32)
    for c in range(NCORES):
        bi, half = c // 2, c % 2
        w0 = half * WPC
        out[bi, w0 : w0 + WPC, :] = res.results[c]["outT"].T
    return out

